# revision 36
# baseline (speedup 1.0000x reference)
"""Trainium2 Bass kernel for nn_EquivariantAttentionLayer.

Reference computation (N=128 frames, P=256 points, D=128, OUT=256, HEADS=16, HD=16):
  qkv  = einsum('ijd,qdhm->qhmij', x, W1)         # temporal QKV
  s1   = einsum('abij,abIj->aiIj', q, k); a1 = softmax(s1, axis=I)
  t    = einsum('aiIj,abIj->abij', a1, v)
  qkv2 = einsum('hmij,qhmgn->qgnij', t, W2)       # point QKV (mix over both head axes)
  s2   = einsum('abij,abiJ->aijJ', q2, k2); a2 = softmax(s2, axis=J)
  pa   = einsum('aijJ,abiJ->ijab', a2, v2).reshape(N,P,256)
  out  = (pa @ fc1_w + fc1_b) @ fc2_w + fc2_b     # NO nonlinearity -> collapses to one 256x256 matmul

Sharding: phase A is point-sharded (temporal attention is independent per point),
phase B/C are frame-sharded (point attention is independent per frame). Two
half-sized AllToAlls re-shard t from point-shards to frame-shards, overlapped
with compute. The FC pair is collapsed on the host (no activation between):
  Wc = fc1_w @ fc2_w ; bc = fc1_b @ fc2_w + fc2_b.
Points are processed in a permuted order (j' = hc*128 + s*16 + jc16); the host
un-permutes the output rows. Heads are processed in PERM order; the host
permutes W2/Wc rows to match.

Engine budget (the kernel is elementwise-bound on ACT+DVE; Pool has no PSUM
port so it cannot help with evictions or exps):
 - x arrives host-staged j-major f16; one DMA-transpose lands x^T (no PE/DVE).
 - phase-A softmax exps are exact on ACT; 7/16 of phase-B exps run on DVE via
   a one-instruction Schraudolph fast-exp (int16(s*128/ln2 + 16256) bitcast to
   bf16), measured end-to-end rel err ~9.7e-3 vs the 2e-2 gate.
 - softmax 1/Z is fused into the PSUM drain (reciprocal + tensor_tensor
   multiply-on-evict, f16 out), which also feeds f16 PE transposes.
 - the FC bias is a K=1 ones-row matmul accumulated into the FC PSUM tile.
 - PSUM evictions round-robin DVE/ACT with stage-dependent ratios (mixes vs
   exp-bound attention stages).
"""

import numpy as np

# ---- problem dims (hardcoded) ----
NF, NP, D = 128, 256, 128       # frames (i/I), points (j/J), input dim
A_, B_ = 16, 16                 # HD (a/g), HEADS (b/n)
F = A_ * B_                     # 256 features
NCORE = 8
PC = NP // NCORE                # 32 points per core (phase A)
HC = PC // 2                    # 16 points per exchange half
NI = NF // NCORE                # 16 frames per core (phase B)
TOK = NF * PC                   # 4096 tokens per core (both phases)

# Head-processing order: batch bh handles PE row groups {2bh, 2bh+1} so that
# same-PSUM-bank score matmuls are always same-group (HW: cross-group same-bank
# PE writes are fatal).
PERM = [4 * (k // 2) + 2 * bh + (k % 2) for bh in range(2) for k in range(8)]

# Point order as seen by phase B / the raw device output (host un-permutes).
JPERM = np.array([s * PC + hc * HC + jc
                  for hc in range(2) for s in range(NCORE) for jc in range(HC)])

# Schraudolph fast-exp constants (bf16 bits as int16): exp(s) ~= bitcast_bf16(
# int16(s * 128/ln2 + 127*128)). Used for half the phase-B softmax exps.
SCH_K = 128.0 / float(np.log(2.0))
SCH_B = 16256.0


def build_program(phases="AB", n_cores=NCORE, reps=1):
    """Build the SPMD Bass program. phases in {"AB", "A", "B"} (A/B for testing).
    reps>1 repeats the whole body (for wall-clock delta timing)."""
    import concourse.bacc as bacc
    import concourse.mybir as mybir
    import concourse.tile as tile
    from concourse.masks import make_identity

    dt = mybir.dt
    f32 = dt.float32
    f32r = dt.float32r
    f16 = dt.float16

    nc = bacc.Bacc(None, target_bir_lowering=False, num_devices=n_cores)

    if "A" in phases:
        x_d = nc.dram_tensor("x", [PC, NF, D], f16, kind="ExternalInput")
        w1qk_d = nc.dram_tensor("w1qk", [D, 8 * 128], f16, kind="ExternalInput")
        w1v_d = nc.dram_tensor("w1v", [D, F], f16, kind="ExternalInput")
    if "B" in phases:
        w2qk_d = nc.dram_tensor("w2qk", [F, 8 * 128], f16, kind="ExternalInput")
        w2v_d = nc.dram_tensor("w2v", [F, F], f16, kind="ExternalInput")
        wc_d = nc.dram_tensor("wc", [F, F], f16, kind="ExternalInput")
        bc_d = nc.dram_tensor("bc", [1, F], f16, kind="ExternalInput")
        out_d = nc.dram_tensor("out", [NI, NP, F], f32, kind="ExternalOutput")

    # exchange buffers (per half): tsh[s, f, il, jc16] = t[f, i=s*NI+il, jc]
    kindA = "ExternalOutput" if phases == "A" else None
    kindB = "ExternalInput" if phases == "B" else None
    tsh_ds = tex_ds = None
    if "A" in phases:
        # A-only builds share one tsh set across reps so outputs don't scale
        # with reps (keeps the paired-timing delta clean)
        n_tsh = reps if phases == "AB" else 1
        tsh_ds = [[nc.dram_tensor(f"tsh{r}_{h}", [NCORE, F, NI, HC], f16,
                                  **({"kind": kindA} if kindA else {}))
                   for h in range(2)] for r in range(n_tsh)]
    if phases == "AB":
        tex_ds = [[nc.dram_tensor(f"tex{r}_{h}", [NCORE, F, NI, HC], f16)
                   for h in range(2)] for r in range(reps)]
    elif phases == "B":
        tex_ds = [[nc.dram_tensor(f"tex0_{h}", [NCORE, F, NI, HC], f16,
                                  kind="ExternalInput") for h in range(2)]]

    with tile.TileContext(nc) as tc:
        with tc.tile_pool(name="consts", bufs=1) as consts:
            # all PE transposes act on f16 data (tu / pa_tok)
            ident = consts.tile([128, 128], f16, tag="ident")
            make_identity(nc, ident[:])

            for r in range(reps):
                if "A" in phases:
                    def do_coll(h, _r=r):
                        if phases != "AB":
                            return
                        nc.gpsimd.collective_compute(
                            "AllToAll", mybir.AluOpType.bypass,
                            replica_groups=[list(range(n_cores))],
                            ins=[tsh_ds[_r][h][:]], outs=[tex_ds[_r][h][:]])
                    _phase_a(nc, tc, tsh_ds[min(r, len(tsh_ds) - 1)], do_coll, x_d, w1qk_d, w1v_d,
                             ident, mybir)
                if "B" in phases:
                    _phase_b(nc, tc, out_d, tex_ds[min(r, len(tex_ds) - 1)],
                             w2qk_d, w2v_d, wc_d, bc_d, ident, mybir)

    nc.compile()
    return nc


def _phase_a(nc, tc, tsh_d, do_coll, x_d, w1qk_d, w1v_d, ident, mybir):
    """Temporal QKV + temporal attention for this core's PC points."""
    dt = mybir.dt
    f32, f16, bf16, f32r = dt.float32, dt.float16, dt.bfloat16, dt.float32r
    Exp = mybir.ActivationFunctionType.Exp
    Copy = mybir.ActivationFunctionType.Copy
    MUL = mybir.AluOpType.mult

    with tc.tile_pool(name="a_sb", bufs=1) as sb, \
         tc.tile_pool(name="a_exp", bufs=3) as expp, \
         tc.tile_pool(name="a_psm", bufs=2, space="PSUM") as psm, \
         tc.tile_pool(name="a_pss", bufs=2, space="PSUM") as pss, \
         tc.tile_pool(name="a_psv", bufs=2, space="PSUM") as psv:

        ec = [0]
        mix_stage = [True]

        def evict(out_ap, in_ap):
            # during the mix stage ACT is idle -> 1:1; during attention ACT
            # carries the exps -> evictions go to DVE
            ec[0] += 1
            on_dve = (ec[0] % 4 < 3) if mix_stage[0] else (ec[0] % 7 < 6)
            if on_dve:
                nc.vector.tensor_copy(out_ap, in_ap)
            else:
                nc.scalar.activation(out_ap, in_ap, Copy)

        w1qk_sb = sb.tile([128, 8 * 128], f16, tag="w1qk")
        nc.sync.dma_start(w1qk_sb[:], w1qk_d[:])
        w1v_sb = sb.tile([128, F], f16, tag="w1v")
        nc.sync.dma_start(w1v_sb[:], w1v_d[:])

        # xt_all[d, j*128+i] = x[i, j, d]; x_d is host-staged j-major f16
        # ([PC, NF, D]) so one DMA-transpose lands the (j, i) token order.
        xt_all = sb.tile([128, TOK], f16, tag="xt")
        nc.sync.dma_start_transpose(
            xt_all[:], x_d[:].rearrange("j i d -> (j i) d"))

        # Q/K, padded head layout: tile c (of 8) holds a in [4c,4c+4), partition
        # 32*(a%4)+b (rows +16..31 zero), free = (j, i). c 0-3 = q, 4-7 = k.
        qk = [sb.tile([128, TOK], f16, tag=f"qk{c}", name=f"qk{c}") for c in range(8)]
        # V^T (+ones col): vt[i, (j, a, 17)]; col 16 of each (j,a) block is 1.0
        vt = sb.tile([128, PC * A_ * 17], bf16, tag="vt")
        nc.gpsimd.memset(vt[:], 1.0)
        vt_r = vt[:].rearrange("p (j a c) -> p j a c", j=PC, a=A_, c=17)
        def mix_half(jh):
            for nt in range(jh * 4, (jh + 1) * 4):
                for c in range(8):
                    ps = psm.tile([128, 512], f32, tag="m")
                    nc.tensor.matmul(ps[:],
                                     w1qk_sb[:, c * 128:(c + 1) * 128],
                                     xt_all[:, nt * 512:(nt + 1) * 512],
                                     start=True, stop=True)
                    evict(qk[c][:, nt * 512:(nt + 1) * 512], ps[:])
                for j in range(nt * 4, (nt + 1) * 4):
                    ps = psv.tile([128, 512], f32, tag="v")
                    nc.tensor.matmul(ps[:, 0:F],
                                     xt_all[:, j * 128:(j + 1) * 128],
                                     w1v_sb[:], start=True, stop=True)
                    evict(vt_r[:, j, :, 0:16],
                          ps[:, 0:F].rearrange("p (a b) -> p a b", a=A_))

        # temporal attention; tu[i, (j, pos, b)] NORMALIZED f16 (pos = PERM'd
        # head order); rz[i, (j, pos)] = 1/Z scratch.
        tu = sb.tile([128, PC * F], f16, tag="tu")
        rz = sb.tile([128, PC * A_], f32, tag="rz")
        # tab[h]: feature-major t; free = (jh, i, jc16)
        tab = [sb.tile([128, TOK], f16, tag=f"tab{h}", name=f"tab{h}")
               for h in range(2)]
        colof = lambda k: (k % 2) * 512 + (k // 2) * 128  # bank = row group

        def emit_scores1(j, bh):
            sps = pss.tile([128, 1024], f32, tag="s", name="sps")
            for k in range(8):
                a = PERM[bh * 8 + k]
                c, s4 = a // 4, a % 4
                tp = (96, 0) if s4 == 3 else None
                # S'[I, i]: lhsT=K (b,I), rhs=Q (b,i)
                nc.tensor.matmul(
                    sps[:, colof(k):colof(k) + 128],
                    qk[4 + c][32 * s4:32 * s4 + 16, j * 128:(j + 1) * 128],
                    qk[c][32 * s4:32 * s4 + 16, j * 128:(j + 1) * 128],
                    start=True, stop=True, tile_position=tp)
            return sps

        def emit_av1(j, bh, sps):
            aex = expp.tile([128, 1024], bf16, tag="aex", name="aex")
            nc.scalar.activation(aex[:], sps[:], Exp)
            tps = psv.tile([128, 512], f32, tag="v", name="tps")
            for k in range(8):
                a = PERM[bh * 8 + k]
                # t^T[i, (b,Z)] = A'^T @ [V^T | 1]
                nc.tensor.matmul(tps[:, k * 17:k * 17 + 17],
                                 aex[:, colof(k):colof(k) + 128],
                                 vt_r[:, j, a, :], start=True, stop=True)
            # drain: reciprocal of Z column, then normalize-on-evict to f16
            tr = tps[:, 0:136].rearrange("p (s c) -> p s c", s=8, c=17)
            o = j * A_ + bh * 8
            nc.vector.reciprocal(rz[:, o:o + 8], tr[:, :, 16])
            rz_b = rz[:, o:o + 8].rearrange("p a -> p a ()") \
                .broadcast_to([128, 8, B_])
            nc.vector.tensor_tensor(
                tu[:, j * F + bh * 128: j * F + bh * 128 + 128]
                  .rearrange("p (a b) -> p a b", a=8),
                tr[:, :, 0:16], rz_b, op=MUL)

        def transpose_group(jh, jq):
            # transpose 4 drained points to feature-major (one batched
            # eviction per h) as soon as their drains are done
            for h in range(2):
                ps = psm.tile([128, 512], f32, tag="m")
                psf = ps[:].bitcast(f16)  # [128, 1024] f16 view
                for q in range(4):
                    j = jh * HC + jq * 4 + q
                    nc.tensor.transpose(
                        psf[:, q * 128:(q + 1) * 128],
                        tu[:, j * F + h * 128: j * F + (h + 1) * 128],
                        ident[:])
                out_ap = tab[h][:, jh * 2048:(jh + 1) * 2048] \
                    .rearrange("p (i j) -> p i j", j=HC)[:, :, jq * 4:jq * 4 + 4] \
                    .rearrange("p i j -> p j i")
                evict(out_ap,
                      psf[:, 0:512].rearrange("p (j i) -> p j i", j=4))

        for jh in range(2):
            mix_stage[0] = True
            mix_half(jh)
            mix_stage[0] = False
            prev = None
            for jc in range(HC):
                j = jh * HC + jc
                for bh in range(2):
                    sps = emit_scores1(j, bh)
                    if prev is not None:
                        emit_av1(prev[0], prev[1], prev[2])
                        if prev[1] == 1 and prev[0] % 4 == 3:
                            transpose_group(jh, (prev[0] - jh * HC) // 4)
                    prev = (j, bh, sps)
            emit_av1(prev[0], prev[1], prev[2])
            transpose_group(jh, HC // 4 - 1)
            for h in range(2):
                nc.sync.dma_start(
                    tsh_d[jh][:, h * 128:(h + 1) * 128, :, :]
                        .rearrange("s f i j -> f s (i j)"),
                    tab[h][:, jh * 2048:(jh + 1) * 2048]
                        .rearrange("p (s ij) -> p s ij", s=NCORE))
            do_coll(jh)


def _phase_b(nc, tc, out_d, tex_d, w2qk_d, w2v_d, wc_d, bc_d, ident, mybir):
    """Point mix + point attention + collapsed FC for this core's NI frames.

    Token order is (il, j') with j' = hc*128 + s*16 + jc16 (host un-permutes)."""
    dt = mybir.dt
    f32, f16, bf16, f32r = dt.float32, dt.float16, dt.bfloat16, dt.float32r
    i16 = dt.int16
    Exp = mybir.ActivationFunctionType.Exp
    Copy = mybir.ActivationFunctionType.Copy
    MUL = mybir.AluOpType.mult
    ADD = mybir.AluOpType.add

    with tc.tile_pool(name="b_sb", bufs=1) as sb, \
         tc.tile_pool(name="b_exp", bufs=3) as expp, \
         tc.tile_pool(name="b_out", bufs=3) as outp, \
         tc.tile_pool(name="b_psm", bufs=2, space="PSUM") as psm, \
         tc.tile_pool(name="b_pss", bufs=2, space="PSUM") as pss, \
         tc.tile_pool(name="b_psv", bufs=2, space="PSUM") as psv:

        ec = [0]
        mix_stage = [True]

        def evict(out_ap, in_ap):
            # mixes: 1:1; attention: DVE carries the Schraudolph exps +
            # drains, so evictions go 5:3 ACT:DVE
            ec[0] += 1
            on_dve = (ec[0] % 2 == 0) if mix_stage[0] else (ec[0] % 8 < 3)
            if on_dve:
                nc.vector.tensor_copy(out_ap, in_ap)
            else:
                nc.scalar.activation(out_ap, in_ap, Copy)

        w2v_sb = sb.tile([128, 2 * F], f16, tag="w2v")  # col block kt = rows kt*128..
        nc.sync.dma_start(w2v_sb[:, 0:F], w2v_d[0:128, :])
        nc.sync.dma_start(w2v_sb[:, F:2 * F], w2v_d[128:256, :])
        wc_sb = sb.tile([128, 2 * F], f16, tag="wc")
        nc.sync.dma_start(wc_sb[:, 0:F], wc_d[0:128, :])
        nc.sync.dma_start(wc_sb[:, F:2 * F], wc_d[128:256, :])
        bias_sb = sb.tile([128, F], f16, tag="bias")
        nc.sync.dma_start(bias_sb[0:1, :], bc_d[:])
        ones_t = sb.tile([128, 128], f16, tag="ones")
        nc.gpsimd.memset(ones_t[:], 1.0)

        # q2/k2 padded head layout; free = (hc, il, s, jc16) = (hc, il, j'128)
        q2k2 = [sb.tile([128, TOK], f16, tag=f"q2k2_{c}", name=f"q2k2_{c}")
                for c in range(8)]
        # v2t[hc][j'_loc, (il, a, 17)]
        v2t = [sb.tile([128, NI * A_ * 17], bf16, tag=f"v2t{h}", name=f"v2t{h}")
               for h in range(2)]
        for h in range(2):
            nc.gpsimd.memset(v2t[h][:], 1.0)
        v2t_r = [v2t[h][:].rearrange("p (i a c) -> p i a c", i=NI, a=A_)
                 for h in range(2)]
        q2k2_r = [q2k2[c][:].rearrange("p (hc il j) -> p hc il j", hc=2, il=NI)
                  for c in range(8)]

        with tc.tile_pool(name="b_t2", bufs=1) as t2p:
            w2qk_sb = [t2p.tile([128, 1024], f16, tag=f"w2qk{kt}",
                                name=f"w2qk{kt}") for kt in range(2)]
            for kt in range(2):
                nc.sync.dma_start(w2qk_sb[kt][:],
                                  w2qk_d[kt * 128:(kt + 1) * 128, :])
            # t2[h][f_local, (hc, il, s, jc16)] — il-major: matmul RHS streams
            # require one flat free dim, which pins the token order. The
            # exchange lands s-major (contiguous 512B-run DMA, fast) in a
            # staging tile; a 4x-rate DVE copy reorders to il-major. A direct
            # il-major DMA would be 2048 descriptors of 32B runs (~7x slower)
            # on the post-collective critical path.
            t2 = [t2p.tile([128, TOK], f16, tag=f"t2_{h}", name=f"t2_{h}")
                  for h in range(2)]
            for hc in range(2):
                for h in range(2):
                    st = t2p.tile([128, 2048], f16, tag="t2s", bufs=2)
                    nc.sync.dma_start(
                        st[:].rearrange("p (s x) -> p s x", s=NCORE),
                        tex_d[hc][:, h * 128:(h + 1) * 128, :, :]
                            .rearrange("s f i j -> f s (i j)"))
                    nc.vector.tensor_copy(
                        t2[h][:, hc * 2048:(hc + 1) * 2048]
                            .rearrange("p (i s j) -> p i s j", i=NI, s=NCORE),
                        st[:].rearrange("p (s i j) -> p i s j", s=NCORE, i=NI))
                # mixes for this half
                for nt in range(4):  # il-quads within the half
                    for c in range(8):
                        ps = psm.tile([128, 512], f32, tag="m")
                        for kt in range(2):
                            nc.tensor.matmul(
                                ps[:],
                                w2qk_sb[kt][:, c * 128:(c + 1) * 128],
                                t2[kt][:, hc * 2048 + nt * 512:
                                       hc * 2048 + (nt + 1) * 512],
                                start=(kt == 0), stop=(kt == 1))
                        evict(q2k2_r[c][:, hc, nt * 4:(nt + 1) * 4, :]
                              .rearrange("p il j -> p (il j)"), ps[:])
                for il in range(NI):
                    ps = psv.tile([128, 512], f32, tag="v")
                    for kt in range(2):
                        nc.tensor.matmul(
                            ps[:, 0:F],
                            t2[kt][:, hc * 2048 + il * 128:
                                   hc * 2048 + (il + 1) * 128],
                            w2v_sb[:, kt * F:(kt + 1) * F],
                            start=(kt == 0), stop=(kt == 1))
                    evict(v2t_r[hc][:, il, :, 0:16],
                          ps[:, 0:F].rearrange("p (a b) -> p a b", a=A_))
        mix_stage[0] = False

        # point attention: pa_tok[jh][j'_loc, (il, pos, n)] NORMALIZED f16
        pa_tok = [sb.tile([128, NI * F], f16, tag=f"pat{jh}", name=f"pat{jh}")
                  for jh in range(2)]
        rz2 = [sb.tile([128, NI * A_], f32, tag=f"rz2_{jh}", name=f"rz2_{jh}")
               for jh in range(2)]
        def emit_scores2(il, bh, m):
            sps = pss.tile([128, 1024], f32, tag="s", name="sps2")
            for kp in range(2):
                k = m * 2 + kp
                a = PERM[bh * 8 + k]
                c, s4 = a // 4, a % 4
                tp = (96, 0) if s4 == 3 else None
                for Jh in range(2):
                    # lhsT=K2 (n, J'_chunk), rhs=Q2 (n, j'=256)
                    nc.tensor.matmul(
                        sps[:, kp * 512 + Jh * 256: kp * 512 + Jh * 256 + 256],
                        q2k2_r[4 + c][32 * s4:32 * s4 + 16, Jh, il, :],
                        q2k2_r[c][32 * s4:32 * s4 + 16, :, il, :],
                        start=True, stop=True, tile_position=tp)
            return sps

        def emit_av2(il, bh, m, sps, tps):
            aex = expp.tile([128, 1024], bf16, tag="aex2", name="aex2")
            if ((il * 8 + bh * 4 + m) * 9) % 16 < 9:
                nc.scalar.activation(aex[:], sps[:], Exp)
            else:
                # Schraudolph fast-exp on DVE: bf16 bits = int16(s*K + B)
                nc.vector.tensor_scalar(aex[:].bitcast(i16), sps[:],
                                        SCH_K, SCH_B, MUL, ADD)
            for kp in range(2):
                k = m * 2 + kp
                a = PERM[bh * 8 + k]
                for jh in range(2):
                    for Jh in range(2):
                        # pa^T[j'_chunk, (n,Z)] = A2'^T @ [V2^T | 1]
                        nc.tensor.matmul(
                            tps[:, (k * 2 + jh) * 17: (k * 2 + jh) * 17 + 17],
                            aex[:, kp * 512 + Jh * 256 + jh * 128:
                                kp * 512 + Jh * 256 + jh * 128 + 128],
                            v2t_r[Jh][:, il, a, :],
                            start=(Jh == 0), stop=(Jh == 1))

        def drain2(il, bh, tps):
            tr = tps[:, 0:272].rearrange("p (s c) -> p s c", s=16, c=17)
            for jh in range(2):
                o = il * A_ + bh * 8
                nc.vector.reciprocal(rz2[jh][:, o:o + 8], tr[:, jh::2, 16])
                rz_b = rz2[jh][:, o:o + 8].rearrange("p a -> p a ()") \
                    .broadcast_to([128, 8, B_])
                nc.vector.tensor_tensor(
                    pa_tok[jh][:, il * F + bh * 128: il * F + bh * 128 + 128]
                        .rearrange("p (a b) -> p a b", a=8),
                    tr[:, jh::2, 0:16], rz_b, op=MUL)

        # per-il tail: transpose to feature-major + FC (+bias via K=1 mm) + store
        def il_tail(il):
            pa_f = sb.tile([128, 512], f16, tag="paf", name="paf", bufs=2)
            ps = psm.tile([128, 512], f32, tag="m")
            psf = ps[:].bitcast(f16)  # [128, 1024] f16 view
            for kt in range(2):
                for jh in range(2):
                    nc.tensor.transpose(
                        psf[:, (kt * 2 + jh) * 128:(kt * 2 + jh) * 128 + 128],
                        pa_tok[jh][:, il * F + kt * 128: il * F + (kt + 1) * 128],
                        ident[:])
            evict(pa_f[:], psf[:, 0:512])
            for jh in range(2):
                ps = psm.tile([128, 512], f32, tag="m")
                for kt in range(2):
                    nc.tensor.matmul(
                        ps[:, 0:F],
                        pa_f[:, (kt * 2 + jh) * 128:(kt * 2 + jh) * 128 + 128],
                        wc_sb[:, kt * F:(kt + 1) * F],
                        start=(kt == 0), stop=False)
                nc.tensor.matmul(ps[:, 0:F], ones_t[0:1, :], bias_sb[0:1, :],
                                 start=False, stop=True)
                ot = outp.tile([128, F], f32, tag="ot")
                evict(ot[:], ps[:, 0:F])
                nc.sync.dma_start(out_d[il, jh * 128:(jh + 1) * 128, :], ot[:])

        prev = None
        tps_map = {}
        for il in range(NI):
            for bh in range(2):
                tps = psv.tile([128, 512], f32, tag="v", name="tps2")
                tps_map[(il, bh)] = tps
                for m in range(4):
                    sps = emit_scores2(il, bh, m)
                    if prev is not None:
                        emit_av2(prev[0], prev[1], prev[2], prev[3],
                                 tps_map[(prev[0], prev[1])])
                        if prev[2] == 3:
                            drain2(prev[0], prev[1], tps_map.pop((prev[0], prev[1])))
                            if prev[1] == 1:
                                il_tail(prev[0])
                    prev = (il, bh, m, sps)
        emit_av2(prev[0], prev[1], prev[2], prev[3], tps_map[(prev[0], prev[1])])
        drain2(prev[0], prev[1], tps_map.pop((prev[0], prev[1])))
        il_tail(prev[0])



# ---------------------------------------------------------------------------
# host side
# ---------------------------------------------------------------------------

def _pad_heads(w, n_in):
    """(n_in, F) with cols f=(a,b) -> (n_in, 4*128): chunk c holds a in
    [4c,4c+4) at col 32*(a%4)+b, cols +16..31 zero."""
    out = np.zeros((n_in, 4 * 128), dtype=np.float32)
    w = w.reshape(n_in, A_, B_)
    for a in range(A_):
        c, s4 = a // 4, a % 4
        out[:, c * 128 + 32 * s4: c * 128 + 32 * s4 + B_] = w[:, a, :]
    return out


def prep_inputs(x, W1, W2, fc1_w, fc1_b, fc2_w, fc2_b):
    """Host-side weight prep + per-core input maps."""
    x = np.asarray(x, dtype=np.float32)
    W1 = np.asarray(W1, dtype=np.float32)
    W2 = np.asarray(W2, dtype=np.float32)
    fc1_w = np.asarray(fc1_w, dtype=np.float32)
    fc1_b = np.asarray(fc1_b, dtype=np.float32)
    fc2_w = np.asarray(fc2_w, dtype=np.float32)
    fc2_b = np.asarray(fc2_b, dtype=np.float32)

    w1q = _pad_heads(W1[0].reshape(D, F), D)
    w1k = _pad_heads(W1[1].reshape(D, F), D)
    w1qk = np.concatenate([w1q, w1k], axis=1).astype(np.float16)
    w1v = W1[2].reshape(D, F).astype(np.float16)

    row_perm = np.array([PERM[pos] * B_ + b for pos in range(A_)
                         for b in range(B_)])
    w2q = _pad_heads(W2[0].reshape(F, F)[row_perm], F)
    w2k = _pad_heads(W2[1].reshape(F, F)[row_perm], F)
    w2qk = np.concatenate([w2q, w2k], axis=1).astype(np.float16)
    w2v = np.ascontiguousarray(W2[2].reshape(F, F)[row_perm]).astype(np.float16)

    wc = np.ascontiguousarray((fc1_w @ fc2_w)[row_perm]).astype(np.float16)
    bc_rep = (fc1_b @ fc2_w + fc2_b).astype(np.float16).reshape(1, F)

    in_maps = []
    for s in range(NCORE):
        # x staged j-major ([PC, NF, D]) in f16 so the device can land the
        # (j, i) token order with a single DMA-transpose.
        xs = np.ascontiguousarray(
            x[:, s * PC:(s + 1) * PC, :].transpose(1, 0, 2)).astype(np.float16)
        in_maps.append({
            "x": xs,
            "w1qk": w1qk, "w1v": w1v,
            "w2qk": w2qk, "w2v": w2v,
            "wc": wc, "bc": bc_rep,
        })
    return in_maps


_CACHE = {}


def kernel(**inputs):
    from concourse.bass_utils import run_bass_kernel_spmd

    in_maps = prep_inputs(**inputs)
    if "nc" not in _CACHE:
        _CACHE["nc"] = build_program("AB", NCORE)
    nc = _CACHE["nc"]
    res = run_bass_kernel_spmd(nc, in_maps, list(range(NCORE)))
    out = np.empty((NF, NP, F), dtype=np.float32)
    for s in range(NCORE):
        out[s * NI:(s + 1) * NI, JPERM, :] = res.results[s]["out"]
    return out

